# revision 1
# baseline (speedup 1.0000x reference)
"""DeltaModel Trainium2 kernel.

Pipeline per core (2 batch elements per core, 8 cores data-parallel):
  embed-gather (one-hot matmul) -> FFN -> LayerNorm -> chunked delta-rule
  fast-weight recurrence (C=128 chunks, WY representation, block-diag
  Jacobi solve + exact block-Horner outer correction) -> readout head.

Math per chunk (K rows = keys token-major [128,64], beta = 1/(||k||^2+eps)):
  A = strict_tril(diag(beta) K K^T)            (A[t,s], t>s)
  X = (I+A)^-1 [K | Kb],  W = X[:,:64], Z = X[:,64:]
  M_c+1^T = M_c^T + K^T W - (K^T Z) M_c^T
  ctx = M_final q ; out = (ctx Wr + br) Wo + bo
(I+A)^-1 applied via: T_bd = (I+A_bd)^-1 (m Jacobi iters, A_bd = 32-block
diagonal part), N = T_bd A_off, then (I+N)^-1 = I-N+N^2-N^3 exactly.
"""

import numpy as np

H = 64
V = 64
B = 16
L = 2048
NCORES = 8
BPC = B // NCORES          # batch per core = 2
NT = 16                    # chunks of 128 tokens per batch element
C = 128                    # chunk length
M_SOLVE = 8                # Jacobi iterations for block-diag solve
PKW = 708
LN_EPS = 1e-5
D_EPS = 1e-6

_CACHE = {}


def _build_nc(legalize=True):
    import concourse.bass as bass
    import concourse.mybir as mybir
    import concourse.tile as tile
    from concourse import masks

    dt = mybir.dt
    f32 = dt.float32
    bf16 = dt.bfloat16
    i32 = dt.int32
    Alu = mybir.AluOpType
    Act = mybir.ActivationFunctionType
    Axis = mybir.AxisListType

    nc = bass.Bass()

    seq_p = nc.declare_dram_parameter("seq", [BPC, L, 2], i32, isOutput=False)
    pk_p = nc.declare_dram_parameter("pk", [128, PKW], f32, isOutput=False)
    out_p = nc.declare_dram_parameter("out", [BPC, V], f32, isOutput=True)

    from contextlib import ExitStack
    with tile.TileContext(nc) as tc, ExitStack() as est:
        persist = est.enter_context(tc.tile_pool(name="persist", bufs=1))
        _tcount = [0]
        def _tile(shape, dtype, name=None):
            n = name or f"t{_tcount[0]}"
            _tcount[0] += 1
            return persist.tile(shape, dtype, name=n, tag=n)
        # ---------- constants ----------
        I64r = _tile([64, 64], f32)
        masks.make_identity(nc, I64r[:])
        I64 = _tile([64, 64], f32)
        nc.vector.tensor_copy(I64[:], I64r[:])
        I128r = _tile([128, 128], f32)
        masks.make_identity(nc, I128r[:])
        I128 = _tile([128, 128], f32)
        nc.vector.tensor_copy(I128[:], I128r[:])
        I128b = _tile([128, 128], bf16)
        nc.vector.tensor_copy(I128b[:], I128r[:])

        # block-diag strict-upper mask (keep S[s,t] with s<t, same 32-block)
        mask_bdsu = _tile([128, 128], f32)
        nc.gpsimd.memset(mask_bdsu[:], 0.0)
        for blk in range(4):
            sub = mask_bdsu[32 * blk:32 * blk + 32, 32 * blk:32 * blk + 32]
            # keep in_ (0) where (p - y) >= 0, else fill 1.0  -> upper strict
            nc.gpsimd.affine_select(
                out=sub, in_=sub, compare_op=Alu.is_ge, fill=1.0,
                base=0, pattern=[[-1, 32]], channel_multiplier=1)

        # off-block strict-lower mask (keep A[t,s] with s<t, different block)
        mask_offsl = _tile([128, 128], f32)
        nc.gpsimd.memset(mask_offsl[:], 1.0)
        # zero everything except strict lower (keep where (p - y) > 0)
        nc.gpsimd.affine_select(
            out=mask_offsl[:], in_=mask_offsl[:], compare_op=Alu.is_gt,
            fill=0.0, base=0, pattern=[[-1, 128]], channel_multiplier=1)
        for blk in range(4):
            nc.gpsimd.memset(
                mask_offsl[32 * blk:32 * blk + 32, 32 * blk:32 * blk + 32], 0.0)

        # row mask: 1 everywhere except partition 127 -> 0 (last key masked)
        rowmask = _tile([128, 1], f32)
        nc.gpsimd.memset(rowmask[:], 1.0)
        nc.gpsimd.affine_select(
            out=rowmask[:], in_=rowmask[:], compare_op=Alu.is_gt, fill=0.0,
            base=127, pattern=[[0, 1]], channel_multiplier=-1)

        iota_i = _tile([64, 1], i32)
        nc.gpsimd.iota(iota_i[:], pattern=[[0, 1]], base=0, channel_multiplier=1)
        iota_f = _tile([64, 1], f32)
        nc.vector.tensor_copy(iota_f[:], iota_i[:])

        ones1x64r = _tile([1, 64], f32)
        nc.gpsimd.memset(ones1x64r[:], 1.0)
        ones1x64 = _tile([1, 64], f32)
        nc.vector.tensor_copy(ones1x64[:], ones1x64r[:])
        ones1x128r = _tile([1, 128], f32)
        nc.gpsimd.memset(ones1x128r[:], 1.0)
        ones1x128 = _tile([1, 128], f32)
        nc.vector.tensor_copy(ones1x128[:], ones1x128r[:])
        one11r = _tile([1, 1], f32)
        nc.gpsimd.memset(one11r[:], 1.0)
        one11 = _tile([1, 1], f32)
        nc.vector.tensor_copy(one11[:], one11r[:])
        epsc = _tile([128, 1], f32)
        nc.gpsimd.memset(epsc[:], LN_EPS)

        # ---------- parameters via one packed DMA ----------
        pk_raw = _tile([128, PKW], f32, name="pk_raw")
        nc.sync.dma_start(pk_raw[:], pk_p[:])
        pk_sb = _tile([128, PKW], f32, name="pk_sb")
        nc.vector.tensor_copy(pk_sb[:], pk_raw[:])
        W2 = pk_sb[:, 0:64]
        W1 = pk_sb[0:64, 64:192]
        emb = pk_sb[0:64, 192:256]
        Wr = pk_sb[0:64, 256:320]
        Wo = pk_sb[0:64, 320:384]
        b1c = pk_sb[:, 384:385]
        gar = pk_sb[0:1, 385:449]
        ber = pk_sb[0:1, 449:513]
        b2r = pk_sb[0:1, 513:577]
        brr = pk_sb[0:1, 577:641]
        bor = pk_sb[0:1, 641:705]

        seqf = []
        for b in range(BPC):
            si = _tile([1, L], i32, name=f"seqi{b}")
            nc.sync.dma_start(si[:], seq_p[b:b + 1, :, 0])
            sf = _tile([1, L], f32, name=f"seqf{b}")
            nc.vector.tensor_copy(sf[:], si[:])
            seqf.append(sf)

        # psum pools
        pp = est.enter_context(tc.tile_pool(name="pp", bufs=2, space="PSUM"))

        # sbuf pools
        sb_kt = est.enter_context(tc.tile_pool(name="sb_kt", bufs=8))
        sb_sbd = est.enter_context(tc.tile_pool(name="sb_sbd", bufs=6))
        sb_x = est.enter_context(tc.tile_pool(name="sb_x", bufs=8))
        sb_v = est.enter_context(tc.tile_pool(name="sb_v", bufs=8))
        sb_fzk = est.enter_context(tc.tile_pool(name="sb_fzk", bufs=8))
        sb_mt = est.enter_context(tc.tile_pool(name="sb_mt", bufs=4))
        sb_sc = est.enter_context(tc.tile_pool(name="sb_sc", bufs=4))
        sb_small = est.enter_context(tc.tile_pool(name="sb_small", bufs=8))

        # broadcast gamma/beta to [128, 64]
        gb_ps = pp.tile([128, H], f32, name="gb_ps", tag="psmall")
        nc.tensor.matmul(gb_ps[:], lhsT=ones1x128[:], rhs=gar, start=True, stop=True)
        gamma_bc = _tile([128, H], f32)
        nc.vector.tensor_copy(gamma_bc[:], gb_ps[:])
        bb_ps = pp.tile([128, H], f32, name="bb_ps", tag="psmall")
        nc.tensor.matmul(bb_ps[:], lhsT=ones1x128[:], rhs=ber, start=True, stop=True)
        beta_bc = _tile([128, H], f32)
        nc.vector.tensor_copy(beta_bc[:], bb_ps[:])

        # ---------- embedding + FFN part 1 (feature-major) ----------
        hT = [_tile([H, L], f32, name=f"hT{b}") for b in range(BPC)]
        g1 = [_tile([2 * H, L], f32, name=f"g1{b}") for b in range(BPC)]
        oh_sb = est.enter_context(tc.tile_pool(name="oh_sb", bufs=4))

        for b in range(BPC):
            for t0 in range(0, L, C):
                bc_ps = pp.tile([V, C], f32, name="bc_ps", tag="psmall")
                nc.tensor.matmul(bc_ps[:], lhsT=ones1x64[:],
                                 rhs=seqf[b][:, t0:t0 + C],
                                 start=True, stop=True)
                oh = oh_sb.tile([V, C], f32, name="oh")
                nc.vector.tensor_scalar(
                    out=oh[:], in0=bc_ps[:], scalar1=iota_f[:], scalar2=None,
                    op0=Alu.is_equal)
                ht_ps = pp.tile([H, C], f32, name="ht_ps", tag="psmall")
                nc.tensor.matmul(ht_ps[:], lhsT=emb, rhs=oh[:],
                                 start=True, stop=True)
                nc.scalar.copy(hT[b][:, t0:t0 + C], ht_ps[:])

            for t0 in range(0, L, 512):
                g_ps = pp.tile([2 * H, 512], f32, name="g_ps", tag="pxg", bufs=1)
                nc.tensor.matmul(g_ps[:], lhsT=W1, rhs=hT[b][:, t0:t0 + 512],
                                 start=True, stop=True)
                nc.vector.tensor_scalar(
                    out=g1[b][:, t0:t0 + 512], in0=g_ps[:], scalar1=b1c,
                    scalar2=0.0, op0=Alu.add, op1=Alu.max)

        # ---------- per 128-token tile: x, LN, keys, chunk math ----------
        R = [_tile([128, 256], bf16, name=f"R{i}") for i in range(BPC * NT)]
        q_row = [_tile([1, H], f32, name=f"q{b}") for b in range(BPC)]
        q_raw = [_tile([1, H], bf16, name=f"qr{b}") for b in range(BPC)]
        mt_cur = [None] * BPC

        for i in range(BPC * NT):
            b, c = i // NT, i % NT
            t0 = c * C
            # x = h + relu(h W1 + b1) W2 + b2   (token-major via PE)
            x_ps = pp.tile([128, H], f32, name="x_ps", tag="pxg2", bufs=1)
            nc.tensor.matmul(x_ps[:], lhsT=g1[b][:, t0:t0 + C], rhs=W2,
                             start=True, stop=False)
            nc.tensor.matmul(x_ps[:], lhsT=hT[b][:, t0:t0 + C], rhs=I64[:],
                             start=False, stop=False)
            nc.tensor.matmul(x_ps[:], lhsT=ones1x128[:], rhs=b2r,
                             start=False, stop=True)
            # LayerNorm
            s1 = sb_small.tile([128, 1], f32, name="s1")
            nc.vector.tensor_reduce(s1[:], x_ps[:], axis=Axis.X, op=Alu.add)
            mu = sb_small.tile([128, 1], f32, name="mu")
            nc.vector.tensor_scalar_mul(mu[:], s1[:], 1.0 / H)
            xc = sb_sc.tile([128, H], f32, name="xc")
            nc.vector.tensor_scalar(out=xc[:], in0=x_ps[:], scalar1=mu[:],
                                    scalar2=None, op0=Alu.subtract)
            sqs = sb_sc.tile([128, H], f32, name="sqs")
            ssq = sb_small.tile([128, 1], f32, name="ssq")
            nc.scalar.activation(sqs[:], xc[:], Act.Square, accum_out=ssq[:])
            sroot = sb_small.tile([128, 1], f32, name="sroot")
            nc.scalar.activation(sroot[:], ssq[:], Act.Sqrt,
                                 bias=epsc[:], scale=1.0 / H)
            rstd = sb_small.tile([128, 1], f32, name="rstd")
            nc.vector.reciprocal(rstd[:], sroot[:])
            kk = sb_sc.tile([128, H], f32, name="kk")
            nc.gpsimd.tensor_scalar(out=kk[:], in0=xc[:], scalar1=rstd[:],
                                    scalar2=None, op0=Alu.mult)
            kg = sb_sc.tile([128, H], f32, name="kg")
            nc.vector.tensor_mul(kg[:], kk[:], gamma_bc[:])
            nc.vector.tensor_add(R[i][:, 0:H], kg[:], beta_bc[:])
            if c == NT - 1:
                # query = last token's normalized h; then mask it out of keys
                nc.sync.dma_start(q_raw[b][:], R[i][127:128, 0:H])
                nc.vector.tensor_copy(q_row[b][:], q_raw[b][:])
                nc.vector.tensor_scalar(
                    out=R[i][:, 0:H], in0=R[i][:, 0:H], scalar1=rowmask[:],
                    scalar2=None, op0=Alu.mult)
            # beta_t and Kb
            ssk = sb_small.tile([128, 1], f32, name="ssk")
            sqk = sb_sc.tile([128, H], f32, name="sqk")
            nc.scalar.activation(sqk[:], R[i][:, 0:H], Act.Square,
                                 accum_out=ssk[:])
            btv = sb_small.tile([128, 1], f32, name="btv")
            nc.vector.tensor_scalar_add(btv[:], ssk[:], D_EPS)
            beta_t = sb_small.tile([128, 1], f32, name="beta_t")
            nc.vector.reciprocal(beta_t[:], btv[:])
            nc.vector.tensor_scalar(out=R[i][:, H:2 * H], in0=R[i][:, 0:H],
                                    scalar1=beta_t[:], scalar2=None,
                                    op0=Alu.mult)

            # ---- transposes ----
            kt_ps = pp.tile([H, 128], f32, name="kt_ps", tag="psmall")
            nc.tensor.matmul(kt_ps[:], lhsT=R[i][:, 0:H], rhs=I128b[:],
                             start=True, stop=True)
            KT = sb_kt.tile([H, 128], bf16, name="KT")
            nc.scalar.copy(KT[:], kt_ps[:])
            kbt_ps = pp.tile([H, 128], f32, name="kbt_ps", tag="psmall")
            nc.tensor.matmul(kbt_ps[:], lhsT=R[i][:, H:2 * H], rhs=I128b[:],
                             start=True, stop=True)
            KbT = sb_kt.tile([H, 128], bf16, name="KbT")
            nc.scalar.copy(KbT[:], kbt_ps[:])

            # ---- S (stationary orientation) and A_off ----
            s_ps = pp.tile([128, 128], f32, name="s_ps", tag="psa", bufs=1)
            nc.tensor.matmul(s_ps[:], lhsT=KT[:], rhs=KbT[:],
                             start=True, stop=True)
            S_bd = sb_sbd.tile([128, 128], bf16, name="S_bd")
            nc.vector.tensor_mul(S_bd[:], s_ps[:], mask_bdsu[:])
            a_ps = pp.tile([128, 128], f32, name="a_ps", tag="psa", bufs=1)
            nc.tensor.matmul(a_ps[:], lhsT=KbT[:], rhs=KT[:],
                             start=True, stop=True)
            nc.vector.tensor_mul(R[i][:, 2 * H:4 * H], a_ps[:], mask_offsl[:])

            # ---- block-diag Jacobi solve: X = R - A_bd X ----
            prev = R[i][:]
            for j in range(M_SOLVE):
                sol_ps = pp.tile([128, 256], f32, name="sol_ps", tag="psol", bufs=3)
                nc.tensor.matmul(sol_ps[:], lhsT=S_bd[:], rhs=prev,
                                 start=True, stop=True)
                X = sb_x.tile([128, 256], bf16, name="X")
                nc.vector.tensor_sub(X[:], R[i][:], sol_ps[:])
                prev = X[:]

            # ---- NT = N^T via PE transpose ----
            nt_ps = pp.tile([128, 128], f32, name="nt_ps", tag="psa", bufs=1)
            nc.tensor.matmul(nt_ps[:], lhsT=prev[:, 2 * H:4 * H], rhs=I128b[:],
                             start=True, stop=True)
            NTt = sb_sbd.tile([128, 128], bf16, name="NTt")
            nc.scalar.copy(NTt[:], nt_ps[:])

            # ---- outer Horner: V = Y - N V  (3x, exact) ----
            Y = prev[:, 0:2 * H]
            prevV = Y
            for j in range(3):
                o_ps = pp.tile([128, 2 * H], f32, name="o_ps", tag="psol", bufs=3)
                nc.tensor.matmul(o_ps[:], lhsT=NTt[:], rhs=prevV,
                                 start=True, stop=True)
                Vt = sb_v.tile([128, 2 * H], bf16, name="Vt")
                nc.vector.tensor_sub(Vt[:], Y, o_ps[:])
                prevV = Vt[:]

            # ---- F = K^T W, ZK = Z^T K ----
            f_ps = pp.tile([H, H], f32, name="f_ps", tag="psmall")
            nc.tensor.matmul(f_ps[:], lhsT=R[i][:, 0:H], rhs=prevV[:, 0:H],
                             start=True, stop=True)
            Ft = sb_fzk.tile([H, H], f32, name="Ft")
            nc.scalar.copy(Ft[:], f_ps[:])
            zk_ps = pp.tile([H, H], f32, name="zk_ps", tag="psmall")
            nc.tensor.matmul(zk_ps[:], lhsT=prevV[:, H:2 * H], rhs=R[i][:, 0:H],
                             start=True, stop=True)
            ZKt = sb_fzk.tile([H, H], f32, name="ZKt")
            nc.scalar.copy(ZKt[:], zk_ps[:])

            # ---- sequential state update ----
            if c == 0:
                mt_cur[b] = Ft
            else:
                sq_ps = pp.tile([H, H], f32, name="sq_ps", tag="psmall")
                nc.tensor.matmul(sq_ps[:], lhsT=ZKt[:], rhs=mt_cur[b][:],
                                 start=True, stop=True)
                tmp = sb_mt.tile([H, H], f32, name="tmp_mt")
                nc.vector.tensor_sub(tmp[:], Ft[:], sq_ps[:])
                mt_new = sb_mt.tile([H, H], f32, name="mt_new")
                nc.vector.tensor_add(mt_new[:], mt_cur[b][:], tmp[:])
                mt_cur[b] = mt_new

        # ---------- readout head ----------
        for b in range(BPC):
            qt_ps = pp.tile([H, 1], f32, name="qt_ps", tag="psmall")
            nc.tensor.matmul(qt_ps[:], lhsT=q_row[b][:], rhs=one11[:],
                             start=True, stop=True)
            qT = sb_small.tile([H, 1], f32, name="qT")
            nc.vector.tensor_copy(qT[:], qt_ps[:])
            cx_ps = pp.tile([H, 1], f32, name="cx_ps", tag="psmall")
            nc.tensor.matmul(cx_ps[:], lhsT=mt_cur[b][:], rhs=qT[:],
                             start=True, stop=True)
            ctx = sb_small.tile([H, 1], f32, name="ctx")
            nc.vector.tensor_copy(ctx[:], cx_ps[:])
            z_ps = pp.tile([H, 1], f32, name="z_ps", tag="psmall")
            nc.tensor.matmul(z_ps[:], lhsT=Wr, rhs=ctx[:],
                             start=True, stop=False)
            nc.tensor.matmul(z_ps[:], lhsT=brr, rhs=one11[:],
                             start=False, stop=True)
            zt = sb_small.tile([H, 1], f32, name="zt")
            nc.vector.tensor_copy(zt[:], z_ps[:])
            y_ps = pp.tile([V, 1], f32, name="y_ps", tag="psmall")
            nc.tensor.matmul(y_ps[:], lhsT=Wo, rhs=zt[:],
                             start=True, stop=False)
            nc.tensor.matmul(y_ps[:], lhsT=bor, rhs=one11[:],
                             start=False, stop=True)
            yt = sb_small.tile([V, 1], f32, name="yt")
            nc.vector.tensor_copy(yt[:], y_ps[:])
            nc.sync.dma_start(out_p[b, :, None], yt[:])

    if legalize:
        _legalize_waits(nc, mybir)
    return nc


def _legalize_waits(nc, mybir):
    """This walrus build encodes at most one sync-wait per instruction.
    Split multi-wait instructions into single-wait NoOp prefixes on the
    same engine (engine queues execute in order, so semantics hold)."""
    k = 0
    for blk in nc.main_func.blocks:
        insts = blk.instructions
        out = []
        changed = False
        for inst in list(insts):
            si = inst.sync_info
            waits = list(si.on_wait) if si is not None and si.on_wait else []
            if len(waits) > 1:
                for w in waits[:-1]:
                    nop = mybir.InstNoOp(name=f"I-wsplit-{k}", ins=[], outs=[])
                    k += 1
                    nop.engine = inst.engine
                    nop.sync_info = mybir.SyncInfo(on_wait=[w], on_update=[])
                    out.append(nop)
                si.on_wait = [waits[-1]]
                changed = True
            out.append(inst)
        if changed:
            while len(insts):
                insts.pop()
            for x in out:
                insts.append(x)


def pack_params(inputs):
    g = lambda k: np.asarray(inputs[k], dtype=np.float32)
    pk = np.zeros((128, PKW), np.float32)
    pk[:, 0:64] = g("W2")
    pk[0:64, 64:192] = g("W1")
    pk[0:64, 192:256] = g("embed")
    pk[0:64, 256:320] = g("Wr")
    pk[0:64, 320:384] = g("Wo")
    pk[:, 384] = g("b1")
    pk[0, 385:449] = g("gamma")
    pk[0, 449:513] = g("beta")
    pk[0, 513:577] = g("b2")
    pk[0, 577:641] = g("br")
    pk[0, 641:705] = g("bo")
    return np.ascontiguousarray(pk)


def _get_nc():
    if "nc" not in _CACHE:
        _CACHE["nc"] = _build_nc()
    return _CACHE["nc"]


def kernel(**inputs):
    from concourse.bass_utils import run_bass_kernel_spmd

    nc = _get_nc()
    seq = np.ascontiguousarray(np.asarray(inputs["seq"], dtype=np.int64))
    seq32 = seq.view(np.int32).reshape(B, L, 2)
    pk = pack_params(inputs)
    in_maps = []
    for core in range(NCORES):
        m = {"seq": np.ascontiguousarray(seq32[core * BPC:(core + 1) * BPC]),
             "pk": pk}
        in_maps.append(m)
    res = run_bass_kernel_spmd(nc, in_maps, core_ids=list(range(NCORES)))
    out = np.concatenate([r["out"] for r in res.results], axis=0)
    return out.astype(np.float32)


if __name__ == "__main__":
    d = np.load("/root/problem/inputs.npz")
    y = kernel(**{k: d[k] for k in d.files})
    o = np.load("/root/problem/oracle.npz")
    rel = np.abs(y - o["y"]).max() / np.abs(o["y"]).max()
    print("Relative error:", rel)

